# revision 32
# baseline (speedup 1.0000x reference)
"""MinGRU Trainium2 kernel.

Problem: x (8, 4096, 1024) fp32; Wz, Wh (1024, 1024); bz, bh (1024,).
    k = x @ Wz.T + bz ; z = sigmoid(k)
    p = x @ Wh.T + bh ; g = where(p >= 0, p + 0.5, sigmoid(p))
    h_t = (1 - z_t) * h_{t-1} + z_t * g_t   (h_0 = 0.5)
The reference computes this recurrence with a log-space parallel scan; here it
is computed directly in linear space (mathematically identical), using the DVE
TensorTensorScanArith instruction along the free axis.

Sharding: data-parallel over batch, one batch element per NeuronCore (8 cores).

Per-core layout: everything lives transposed, H on partitions, S on the free
axis.  The two GEMMs run in fp8-e4m3 DoubleRow perf mode (2 fp8 k-rows per PE
pass): x and the weights are quantized host-side (x*16, W*512, both well
inside e4m3's +-240 normal range) and the combined 1/8192 scale is folded into
the ScalarE activation `scale` operand.  k/p tiles (128, 512) come out of PSUM
from 4 DoubleRow matmuls (256-deep contraction each).

Post-GEMM per (strip, m) unit, balanced across the non-PE engines so the PE
matmul stream stays the bottleneck:
  ScalarE: z = sigmoid(s*kp + bz); sp = sigmoid(s*pp + bh);
           rp = copy(s*pp + bh + 0.5)          (all bf16 out)
  DVE:     a = 1 - z; g = max(sp, rp)  [== where(p>=0, p+0.5, sigmoid(p))];
           b = z * g                           (bf16, packed 2x/4x modes)
  DVE/GpSimd (alternating by m): h = scan(a, b) along the free axis
Output h is stored bf16 and upcast on the host.
"""

import os
import sys

import numpy as np

for _p in ("/opt/trn_rl_repo", "/root/.axon_site/_ro/trn_rl_repo"):
    if os.path.isdir(_p) and _p not in sys.path:
        sys.path.insert(0, _p)

import ml_dtypes  # noqa: E402

import concourse.bass as bass  # noqa: E402
import concourse.mybir as mybir  # noqa: E402
import concourse.tile as tile  # noqa: E402
from concourse import bacc  # noqa: E402
from concourse.bass_utils import run_bass_kernel_spmd  # noqa: E402

F32 = mybir.dt.float32
F32R = mybir.dt.float32r  # fp32 bits, full-rate PE streaming mode
BF16 = mybir.dt.bfloat16
F8 = mybir.dt.float8e4  # TRN e4m3 (max +-240)
E4M3 = ml_dtypes.float8_e4m3
N_CORES = 8
B, S, D, H = 8, 4096, 1024, 1024
TS = 512  # sequence strip width (= one PSUM bank of fp32)
NJ = D // 256  # DoubleRow pair count (256-deep contraction per matmul)
NM = H // 128
SX = 16.0  # x quant scale (|x| < ~6 -> < 96)
SW = 512.0  # weight quant scale (|W| <= 1/32 -> <= 16)
SCALE = 1.0 / (SX * SW)  # de-scale folded into the ScalarE activations

_cache: dict = {}


def build_nc(seq_len: int = S, n_cores: int = N_CORES):
    """Build and compile the per-core Bass module (SPMD, identical program)."""
    nt = seq_len // TS
    nc = bacc.Bacc(
        "TRN2", target_bir_lowering=False, debug=False, num_devices=n_cores
    )

    xq_d = nc.dram_tensor("xq", [D, seq_len], F8, kind="ExternalInput")
    wz_d = nc.dram_tensor("wz8", [D, H], F8, kind="ExternalInput")
    wh_d = nc.dram_tensor("wh8", [D, H], F8, kind="ExternalInput")
    bz_d = nc.dram_tensor("bz", [H], F32, kind="ExternalInput")
    bh_d = nc.dram_tensor("bh", [H], F32, kind="ExternalInput")
    hT_d = nc.dram_tensor("hT", [H, seq_len], BF16, kind="ExternalOutput")

    # DoubleRow pair views: d = j*256 + i*128 + p -> [p, j, i, cols]
    xq4 = xq_d.ap().rearrange("(j two p) s -> p j two s", j=NJ, two=2, p=128)
    wz4 = wz_d.ap().rearrange("(j two p) h -> p j two h", j=NJ, two=2, p=128)
    wh4 = wh_d.ap().rearrange("(j two p) h -> p j two h", j=NJ, two=2, p=128)

    AF = mybir.ActivationFunctionType
    OP = mybir.AluOpType
    DR = mybir.MatmulPerfMode.DoubleRow

    with tile.TileContext(nc) as tc:
        with (
            tc.tile_pool(name="singles", bufs=1) as singles,
            tc.tile_pool(name="xs", bufs=3) as xpool,
            tc.tile_pool(name="work", bufs=10) as work,
            tc.tile_pool(name="hbuf", bufs=3) as hpool,
            tc.tile_pool(name="psum", bufs=4, space="PSUM") as psum,
        ):
            # PE warm-up: the HAM clock gate holds the PE at 1.2 GHz until it
            # has been busy ~3.4 us.  The PE sits idle anyway while the first
            # DMAs land, so burn that time on dummy matmuls over a zeroed
            # tile — the first real matmuls then run at 2.4 GHz.
            warm = singles.tile([128, TS], F32, tag="warm")
            nc.gpsimd.memset(warm[:], 0.0)
            wps = psum.tile([128, TS], F32, tag="kp")
            for i in range(12):
                nc.tensor.matmul(
                    wps[:], lhsT=warm[:, :128].bitcast(F32R),
                    rhs=warm[:].bitcast(F32R),
                    start=(i == 0), stop=(i == 11),
                )
            # Biases first: they are tiny but gate every activation (and the
            # activations drain PSUM for the PE), so they must not queue
            # behind the weights on the serialized DMA stream.
            bz_sb = singles.tile([128, NM], F32, tag="bz")
            nc.sync.dma_start(out=bz_sb, in_=bz_d.ap().rearrange("(m p) -> p m", p=128))
            bh_sb = singles.tile([128, NM], F32, tag="bh")
            nc.sync.dma_start(out=bh_sb, in_=bh_d.ap().rearrange("(m p) -> p m", p=128))
            # bh5 = bh + 0.5 (bias for the linear branch of g); bzn = -bz
            # (bias for computing a = sigmoid(-k) directly)
            bh5_sb = singles.tile([128, NM], F32, tag="bh5")
            nc.vector.tensor_scalar(
                out=bh5_sb, in0=bh_sb, scalar1=0.5, scalar2=None, op0=OP.add
            )
            bzn_sb = singles.tile([128, NM], F32, tag="bzn")
            nc.vector.tensor_scalar(
                out=bzn_sb, in0=bz_sb, scalar1=-1.0, scalar2=None, op0=OP.mult
            )
            # Initial DMA order: x strip 0 first (smallest prefix the first
            # kp group needs), then each weight tensor as one 1 MB transfer
            # (fewer serialized issue ops on the Sync queue).
            xs0 = xpool.tile([128, NJ, 2, TS], F8, tag="xs")
            nc.sync.dma_start(out=xs0, in_=xq4[:, :, :, 0:TS])
            wz_sb = singles.tile([128, NJ, 2, H], F8, tag="wz")
            nc.sync.dma_start(out=wz_sb, in_=wz4)
            wh_sb = singles.tile([128, NJ, 2, H], F8, tag="wh")
            nc.sync.dma_start(out=wh_sb, in_=wh4)
            # The last 512-wide strip is split in two 256-wide strips: the
            # end-of-kernel pipeline drain runs on half-width tiles, halving
            # the post-matmul tail.
            strips = [(s * TS, TS) for s in range(nt - 1)]
            strips += [((nt - 1) * TS, TS // 2), ((nt - 1) * TS + TS // 2, TS // 2)]
            h_prev: list = [None] * NM

            def post_gemm(m, kp, pp, tw, hstrip):
                """Gate math + scan for one (strip, m) unit.  Everything flows
                strictly PE -> ScalarE -> DVE (no GpSimd, no back-edges), so
                no engine queue head ever waits on a slower lateral engine."""
                z = work.tile([128, TS], BF16, tag="z")
                nc.scalar.activation(
                    out=z[:, :tw], in_=kp[:, :tw], func=AF.Sigmoid,
                    bias=bz_sb[:, m:m + 1], scale=SCALE,
                )
                sp = work.tile([128, TS], BF16, tag="sp")
                nc.scalar.activation(
                    out=sp[:, :tw], in_=pp[:, :tw], func=AF.Sigmoid,
                    bias=bh_sb[:, m:m + 1], scale=SCALE,
                )
                rp = work.tile([128, TS], BF16, tag="rp")
                nc.scalar.activation(
                    out=rp[:, :tw], in_=pp[:, :tw], func=AF.Identity,
                    bias=bh5_sb[:, m:m + 1], scale=SCALE,
                )
                # a = 1 - z: alternate between a 4th ScalarE activation
                # (sigmoid(-(k+bz)), even units) and a packed DVE
                # tensor_scalar (odd units) — the DVE is the saturated
                # engine, ScalarE has headroom for half the a's
                a = work.tile([128, TS], BF16, tag="a")
                if m % 2 == 0:
                    nc.scalar.activation(
                        out=a[:, :tw], in_=kp[:, :tw], func=AF.Sigmoid,
                        bias=bzn_sb[:, m:m + 1], scale=-SCALE,
                    )
                else:
                    nc.vector.tensor_scalar(
                        out=a[:, :tw], in0=z[:, :tw], scalar1=-1.0,
                        scalar2=1.0, op0=OP.mult, op1=OP.add,
                    )
                # g = max(sigmoid(p+bh), p+bh+0.5) == where(p+bh>=0, ., .)
                g = work.tile([128, TS], BF16, tag="g")
                nc.vector.tensor_tensor(
                    out=g[:, :tw], in0=sp[:, :tw], in1=rp[:, :tw], op=OP.max
                )
                # b = z * g (packed 2-elem/cycle tensor_tensor)
                b = work.tile([128, TS], BF16, tag="b")
                nc.vector.tensor_tensor(
                    out=b[:, :tw], in0=z[:, :tw], in1=g[:, :tw], op=OP.mult
                )
                # h_t = a_t * h_{t-1} + b_t along the free axis
                if h_prev[m] is None:
                    init = 0.5
                else:
                    pt, pw = h_prev[m]
                    init = pt[:, m, pw - 1:pw]
                nc.vector.tensor_tensor_scan(
                    out=hstrip[:, m, :tw], data0=a[:, :tw], data1=b[:, :tw],
                    initial=init, op0=OP.mult, op1=OP.add,
                )
                h_prev[m] = (hstrip, tw)

            # x tiles are prefetched one strip ahead so their issue sits in
            # front of the (scan-gated) h-store in the Sync queue
            xs_tiles: dict = {0: xs0}

            def load_xs(s):
                if s >= len(strips) or s in xs_tiles:
                    return
                ts0, tw = strips[s]
                xt = xpool.tile([128, NJ, 2, TS], F8, tag="xs")
                nc.sync.dma_start(
                    out=xt[:, :, :, :tw], in_=xq4[:, :, :, ts0:ts0 + tw]
                )
                xs_tiles[s] = xt

            load_xs(1)
            for s, (ts0, tw) in enumerate(strips):
                ts_sl = slice(ts0, ts0 + tw)
                load_xs(s + 1)
                load_xs(s + 2)
                xs = xs_tiles.pop(s)
                hstrip = hpool.tile([128, NM, TS], BF16, tag="h")
                for m in range(NM):
                    m_sl = slice(m * 128, (m + 1) * 128)
                    kp = psum.tile([128, TS], F32, tag="kp")
                    pp = psum.tile([128, TS], F32, tag="pp")
                    for j in range(NJ):
                        nc.tensor.matmul(
                            kp[:, :tw],
                            lhsT=wz_sb[:, j, :, m_sl],
                            rhs=xs[:, j, :, :tw],
                            start=(j == 0),
                            stop=(j == NJ - 1),
                            perf_mode=DR,
                        )
                    for j in range(NJ):
                        nc.tensor.matmul(
                            pp[:, :tw],
                            lhsT=wh_sb[:, j, :, m_sl],
                            rhs=xs[:, j, :, :tw],
                            start=(j == 0),
                            stop=(j == NJ - 1),
                            perf_mode=DR,
                        )
                    post_gemm(m, kp, pp, tw, hstrip)
                nc.sync.dma_start(
                    out=hT_d.ap()[:, ts_sl].rearrange("(m p) s -> p m s", p=128),
                    in_=hstrip[:, :, :tw],
                )

    nc.compile()
    return nc


def quantize_inputs(x, Wz, bz, Wh, bh):
    """Host-side prep shared by kernel() and the test harness: returns the
    per-core input maps (x transposed + fp8-quantized, weights fp8)."""
    x = np.ascontiguousarray(x, dtype=np.float32)
    wz8 = (np.ascontiguousarray(Wz.T, dtype=np.float32) * SW).astype(E4M3)
    wh8 = (np.ascontiguousarray(Wh.T, dtype=np.float32) * SW).astype(E4M3)
    bz = np.ascontiguousarray(bz, dtype=np.float32)
    bh = np.ascontiguousarray(bh, dtype=np.float32)
    return [
        {
            "xq": (x[b].T * SX).astype(E4M3),
            "wz8": wz8,
            "wh8": wh8,
            "bz": bz,
            "bh": bh,
        }
        for b in range(x.shape[0])
    ]


def kernel(x, Wz, bz, Wh, bh):
    key = "nc"
    if key not in _cache:
        _cache[key] = build_nc()
    nc = _cache[key]

    in_maps = quantize_inputs(x, Wz, bz, Wh, bh)
    res = run_bass_kernel_spmd(nc, in_maps, list(range(N_CORES)))
    out = np.empty((B, S, H), np.float32)
    for b in range(N_CORES):
        out[b] = res.results[b]["hT"].astype(np.float32).T
    return out
